# revision 7
# baseline (speedup 1.0000x reference)
"""Multi-head attention TRN2 Bass kernel (8 NeuronCores, tensor-parallel).

Sharding: Megatron-style TP over (batch x head-group). 8 cores = 2 batches x 4
head-groups of 4 heads each. Each core computes its heads' Q/K/V projections,
masked-softmax attention, and a partial output projection; the host sums the 4
partials per batch (the TP unshard).

Per-core kernel layout:
  qwT/kwT: (d_local=256, M=2048)   [d on partitions, 2 SBUF tensors of 128]
  vw:      (N=2048, d_local)       [n on partitions, +ones col per head]
  scoresT: (n-tile=128, m)  on PE -> PSUM (f32r matmuls, two heads packed
                                    into PE row-groups 0-1 / 2-3)
  exp:     ScalarE from PSUM, scale=1/8 folded in, -> fp16 SBUF
           (max-subtraction skipped: |scores|/8 <= ~6 so exp can't overflow)
  mask:    VectorE fp16 multiply by keep=(1-mask).T  (2x DVE mode)
  ctx:     PE fp16, lhsT=[vw_h | 1] (M=65) -> row 64 accumulates softmax sums
  norm:    approx-reciprocal + PE ones-broadcast, folded into PSUM->SBUF copy
  out:     partial (2048, 1024) = ctxT.T @ WoT_local
"""
import sys

for p in ("/opt/trn_rl_repo",):
    if p not in sys.path:
        sys.path.insert(0, p)

from contextlib import ExitStack

import numpy as np

import concourse.bass as bass
import concourse.tile as tile
from concourse import bacc, mybir
from concourse.bass_utils import run_bass_kernel_spmd

F32 = mybir.dt.float32
F32R = mybir.dt.float32r
F16 = mybir.dt.float16
EXP = mybir.ActivationFunctionType.Exp

B, M, N, E = 2, 2048, 2048, 1024  # batch, q-len, k-len, d_model
H, DK = 16, 64                    # heads, head dim
NCORES = 8
GROUPS = 4                        # head groups (cores per batch)
DLOC = (H // GROUPS) * DK         # 256 per-core projection width
HL = H // GROUPS                  # 4 local heads
ET = E // 128                     # 8 e-tiles
NT = N // 128                     # 16 n-tiles
VSTR = HL * (DK + 1)              # 260: vw slot stride per n-tile


def build_program() -> bass.Bass:
    nc = bacc.Bacc()

    qT_d = nc.dram_tensor("qT", [E, M], F32R, kind="ExternalInput")
    kT_d = nc.dram_tensor("kT", [E, N], F32R, kind="ExternalInput")
    vT_d = nc.dram_tensor("vT", [E, N], F16, kind="ExternalInput")
    keepT_d = nc.dram_tensor("keepT", [N, M], F16, kind="ExternalInput")
    wqT_d = nc.dram_tensor("wqT", [E, DLOC], F32R, kind="ExternalInput")
    wkT_d = nc.dram_tensor("wkT", [E, DLOC], F32R, kind="ExternalInput")
    wvT_d = nc.dram_tensor("wvT", [E, DLOC], F16, kind="ExternalInput")
    woT_d = nc.dram_tensor("woT", [DLOC, E], F32R, kind="ExternalInput")
    out_d = nc.dram_tensor("out", [M, E], F32, kind="ExternalOutput")

    with tile.TileContext(nc) as tc, ExitStack() as ctx:
        const_pool = ctx.enter_context(tc.tile_pool(name="const", bufs=1))
        w_pool = ctx.enter_context(tc.tile_pool(name="weights", bufs=1))
        act_pool = ctx.enter_context(tc.tile_pool(name="acts", bufs=1))

        ones64_f = const_pool.tile([1, 64], F32)
        nc.vector.memset(ones64_f[:], 1.0)
        ones64 = const_pool.tile([1, 64], F32R)
        nc.vector.tensor_copy(ones64[:], ones64_f[:])

        wq_sb = w_pool.tile([128, ET * DLOC], F32R, tag="wq")
        wk_sb = w_pool.tile([128, ET * DLOC], F32R, tag="wk")
        wv_sb = w_pool.tile([128, ET * DLOC], F16, tag="wv")
        wo_sb = w_pool.tile([128, 2 * E], F32R, tag="wo")
        for et in range(ET):
            sl = bass.ts(et, DLOC)
            nc.sync.dma_start(wq_sb[:, sl], wqT_d[bass.ts(et, 128), :])
            nc.sync.dma_start(wk_sb[:, sl], wkT_d[bass.ts(et, 128), :])
            nc.sync.dma_start(wv_sb[:, sl], wvT_d[bass.ts(et, 128), :])
        for kt in range(2):
            nc.sync.dma_start(wo_sb[:, bass.ts(kt, E)], woT_d[bass.ts(kt, 128), :])

        qw_sb = [act_pool.tile([128, M], F32R, tag=f"qw{i}", name=f"qw{i}") for i in range(2)]
        kw_sb = [act_pool.tile([128, N], F32R, tag=f"kw{i}", name=f"kw{i}") for i in range(2)]
        vw_sb = act_pool.tile([128, NT * VSTR], F16, tag="vw")
        ctx_sb = [act_pool.tile([128, M], F32R, tag=f"ctx{i}", name=f"ctx{i}") for i in range(2)]
        nc.vector.memset(vw_sb[:], 1.0)  # pre-fill ones cols; data cols overwritten

        # ---- projections ----
        with (
            tc.tile_pool(name="xT", bufs=3) as xT_pool,
            tc.tile_pool(name="proj_ps", bufs=8, space="PSUM") as pps,
        ):
            def proj_qk(xT_dram, w_sb, out_t):
                ps = [pps.tile([128, 512], F32, tag="pp", name=f"pp{j2}") for j2 in range(8)]
                for et in range(ET):
                    xt = xT_pool.tile([128, M], F32R, tag="xt")
                    nc.sync.dma_start(xt[:], xT_dram[bass.ts(et, 128), :])
                    for d2 in range(2):
                        for mc in range(4):
                            nc.tensor.matmul(
                                ps[d2 * 4 + mc][:],
                                w_sb[:, et * DLOC + d2 * 128 : et * DLOC + (d2 + 1) * 128],
                                xt[:, bass.ts(mc, 512)],
                                start=(et == 0), stop=(et == ET - 1),
                            )
                for d2 in range(2):
                    for mc in range(4):
                        nc.vector.tensor_copy(
                            out_t[d2][:, bass.ts(mc, 512)], ps[d2 * 4 + mc][:]
                        )

            proj_qk(qT_d, wq_sb, qw_sb)
            proj_qk(kT_d, wk_sb, kw_sb)

            # v projection: vw (n-tile, d_local) in 2 groups of 8 n-tiles
            for g in range(2):
                ps = [pps.tile([128, 512], F32, tag="pp", name=f"pp{j2}") for j2 in range(8)]
                for et in range(ET):
                    vt = xT_pool.tile([128, N], F16, tag="vt")
                    nc.sync.dma_start(vt[:], vT_d[bass.ts(et, 128), :])
                    for j in range(8):
                        nt = g * 8 + j
                        nc.tensor.matmul(
                            ps[j][:, 0:DLOC],
                            vt[:, bass.ts(nt, 128)],
                            wv_sb[:, bass.ts(et, DLOC)],
                            start=(et == 0), stop=(et == ET - 1),
                        )
                for j in range(8):
                    nt = g * 8 + j
                    for h in range(HL):
                        nc.vector.tensor_copy(
                            vw_sb[:, nt * VSTR + h * 65 : nt * VSTR + h * 65 + 64],
                            ps[j][:, bass.ts(h, 64)],
                        )

        # ---- attention ----
        with (
            tc.tile_pool(name="keep", bufs=1) as keep_pool,
            tc.tile_pool(name="s_ps", bufs=3, space="PSUM") as s_ps,
            tc.tile_pool(name="c_ps", bufs=1, space="PSUM") as c_ps,
            tc.tile_pool(name="attn", bufs=6) as attn_pool,
            tc.tile_pool(name="attnm", bufs=6) as attnm_pool,
            tc.tile_pool(name="eps", bufs=2) as eps_pool,
        ):
            for mh in range(2):  # m-halves of 1024
                moff = mh * 1024
                keep_sb = keep_pool.tile([128, NT * 1024], F16, tag="keep")
                for nt in range(NT):
                    nc.sync.dma_start(
                        keep_sb[:, bass.ts(nt, 1024)],
                        keepT_d[bass.ts(nt, 128), moff : moff + 1024],
                    )
                for h in range(HL):  # local heads, one at a time
                    hp, hl = divmod(h, 2)
                    pctx = c_ps.tile([65, 1024], F32, tag="pctx")
                    # software pipeline: ctx(nt - DEPTH) issues after
                    # scores/exp/mask(nt) so PE always has ready work
                    DEPTH = 2
                    ams = {}
                    for step in range(NT + DEPTH):
                        if step < NT:
                            nt = step
                            ps = s_ps.tile([128, 1024], F32, tag="ps")
                            for mc2 in range(2):
                                nc.tensor.matmul(
                                    ps[:, bass.ts(mc2, 512)],
                                    kw_sb[hp][bass.ts(hl, 64), bass.ts(nt, 128)],
                                    qw_sb[hp][
                                        bass.ts(hl, 64),
                                        moff + mc2 * 512 : moff + (mc2 + 1) * 512,
                                    ],
                                    start=True, stop=True,
                                )
                            au = attn_pool.tile([128, 1024], F16, tag="au")
                            nc.scalar.activation(au[:], ps[:], EXP, scale=0.125)
                            am = attnm_pool.tile([128, 1024], F16, tag="am")
                            nc.vector.tensor_mul(
                                am[:], au[:], keep_sb[:, bass.ts(nt, 1024)]
                            )
                            ams[nt] = am
                        if step >= DEPTH:
                            nt = step - DEPTH
                            am = ams.pop(nt)
                            for mc2 in range(2):
                                nc.tensor.matmul(
                                    pctx[:, bass.ts(mc2, 512)],
                                    vw_sb[:, nt * VSTR + h * 65 : nt * VSTR + (h + 1) * 65],
                                    am[:, bass.ts(mc2, 512)],
                                    start=(nt == 0), stop=(nt == NT - 1),
                                )
                    # normalize: ctxT = pctx[0:64] * (1 / sums) ; sums = row 64
                    sums = eps_pool.tile([1, 1024], F32R, tag="sums")
                    nc.vector.tensor_copy(sums[:], pctx[64:65, :])
                    prb = s_ps.tile([128, 1024], F32, tag="ps")
                    for mc2 in range(2):
                        nc.tensor.matmul(
                            prb[0:64, bass.ts(mc2, 512)],
                            ones64[:],
                            sums[:, bass.ts(mc2, 512)],
                            start=True, stop=True,
                        )
                    rbs = eps_pool.tile([64, 1024], F32, tag="rbs")
                    nc.vector.reciprocal_approx_fast(rbs[:], prb[0:64, :])
                    nc.vector.tensor_mul(
                        ctx_sb[hp][bass.ts(hl, 64), moff : moff + 1024],
                        pctx[0:64, :],
                        rbs[:],
                    )

        # ---- output projection (partial) ----
        with (
            tc.tile_pool(name="o_ps", bufs=3, space="PSUM") as o_ps,
            tc.tile_pool(name="o_sb", bufs=3) as o_sb_pool,
        ):
            for mt in range(M // 128):
                for ec in range(2):
                    po = o_ps.tile([128, 512], F32, tag="po")
                    for kt in range(2):
                        nc.tensor.matmul(
                            po[:],
                            ctx_sb[kt][:, bass.ts(mt, 128)],
                            wo_sb[:, kt * E + ec * 512 : kt * E + (ec + 1) * 512],
                            start=(kt == 0), stop=(kt == 1),
                        )
                    ob = o_sb_pool.tile([128, 512], F32, tag="ob")
                    nc.vector.tensor_copy(ob[:], po[:])
                    nc.sync.dma_start(
                        out_d[bass.ts(mt, 128), bass.ts(ec, 512)], ob[:]
                    )

    nc.finalize()
    return nc


_PROGRAM = None


def _get_program():
    global _PROGRAM
    if _PROGRAM is None:
        _PROGRAM = build_program()
    return _PROGRAM


def _make_in_maps(q, k, v, mask, Wq, Wk, Wv, Wo):
    q = np.asarray(q, dtype=np.float32)
    k = np.asarray(k, dtype=np.float32)
    v = np.asarray(v, dtype=np.float32)
    mask = np.asarray(mask)
    Wq = np.asarray(Wq, dtype=np.float32)
    Wk = np.asarray(Wk, dtype=np.float32)
    Wv = np.asarray(Wv, dtype=np.float32)
    Wo = np.asarray(Wo, dtype=np.float32)

    per_batch = {}
    for b in range(B):
        per_batch[b] = dict(
            qT=np.ascontiguousarray(q[b].T),
            kT=np.ascontiguousarray(k[b].T),
            vT=np.ascontiguousarray(v[b].T.astype(np.float16)),
            keepT=np.ascontiguousarray(
                np.logical_not(mask[b]).T.astype(np.float16)
            ),
        )

    in_maps = []
    for c in range(NCORES):
        b, hg = divmod(c, GROUPS)
        sl = slice(hg * DLOC, (hg + 1) * DLOC)
        in_maps.append(
            dict(
                per_batch[b],
                wqT=np.ascontiguousarray(Wq[sl].T),
                wkT=np.ascontiguousarray(Wk[sl].T),
                wvT=np.ascontiguousarray(Wv[sl].T.astype(np.float16)),
                woT=np.ascontiguousarray(Wo[:, sl].T),
            )
        )
    return in_maps


def _run(in_maps, trace=False):
    nc = _get_program()
    return run_bass_kernel_spmd(
        nc, in_maps, list(range(NCORES)), trace=trace
    )


def _assemble(results):
    out = np.zeros((B, M, E), dtype=np.float32)
    for c in range(NCORES):
        b = c // GROUPS
        out[b] += results[c]["out"]
    return out


def kernel(q, k, v, mask, Wq, Wk, Wv, Wo):
    in_maps = _make_in_maps(q, k, v, mask, Wq, Wk, Wv, Wo)
    res = _run(in_maps, trace=False)
    return _assemble(res.results)


def run_profiled(q, k, v, mask, Wq, Wk, Wv, Wo):
    """Like kernel(), but traces execution; returns (out, BassKernelResults)."""
    in_maps = _make_in_maps(q, k, v, mask, Wq, Wk, Wv, Wo)
    res = _run(in_maps, trace=True)
    return _assemble(res.results), res


# revision 9
# speedup vs baseline: 1.0845x; 1.0845x over previous
"""Multi-head attention TRN2 Bass kernel (8 NeuronCores, tensor-parallel).

Sharding: Megatron-style TP over (batch x head-group). 8 cores = 2 batches x 4
head-groups of 4 heads each. Each core computes its heads' Q/K/V projections,
masked-softmax attention, and a partial output projection; the host sums the 4
partials per batch (the TP unshard).

Per-core kernel layout:
  qwT/kwT: (d_local=256, M=2048)   [d on partitions, 2 SBUF tensors of 128]
  vw:      (N=2048, d_local)       [n on partitions, +ones col per head]
  scoresT: (n-tile=128, m)  on PE -> PSUM (f32r matmuls, two heads packed
                                    into PE row-groups 0-1 / 2-3)
  exp:     ScalarE from PSUM, scale=1/8 folded in, -> fp16 SBUF
           (max-subtraction skipped: |scores|/8 <= ~6 so exp can't overflow)
  mask:    VectorE fp16 multiply by keep=(1-mask).T  (2x DVE mode)
  ctx:     PE fp16, lhsT=[vw_h | 1] (M=65) -> row 64 accumulates softmax sums
  norm:    approx-reciprocal + PE ones-broadcast, folded into PSUM->SBUF copy
  out:     partial (2048, 1024) = ctxT.T @ WoT_local
"""
import os
import sys

for p in ("/opt/trn_rl_repo",):
    if p not in sys.path:
        sys.path.insert(0, p)

from contextlib import ExitStack

import numpy as np

import concourse.bass as bass
import concourse.tile as tile
from concourse import bacc, mybir
from concourse.bass_utils import run_bass_kernel_spmd

F32 = mybir.dt.float32
F32R = mybir.dt.float32r
F16 = mybir.dt.float16
EXP = mybir.ActivationFunctionType.Exp

B, M, N, E = 2, 2048, 2048, 1024  # batch, q-len, k-len, d_model
H, DK = 16, 64                    # heads, head dim
NCORES = 8
GROUPS = 4                        # head groups (cores per batch)
DLOC = (H // GROUPS) * DK         # 256 per-core projection width
HL = H // GROUPS                  # 4 local heads
ET = E // 128                     # 8 e-tiles
NT = N // 128                     # 16 n-tiles
VSTR = HL * (DK + 1)              # 260: vw slot stride per n-tile

# tunables (env-overridable for experiments)
DEPTH = int(os.environ.get("K_DEPTH", "2"))
S_BUFS = int(os.environ.get("K_SBUFS", "3"))
C_BUFS = int(os.environ.get("K_CBUFS", "1"))
AU_BUFS = int(os.environ.get("K_AUBUFS", "6"))
AM_BUFS = int(os.environ.get("K_AMBUFS", "6"))


def build_program() -> bass.Bass:
    nc = bacc.Bacc()

    qT_d = nc.dram_tensor("qT", [E, M], F16, kind="ExternalInput")
    kT_d = nc.dram_tensor("kT", [E, N], F16, kind="ExternalInput")
    vT_d = nc.dram_tensor("vT", [E, N], F16, kind="ExternalInput")
    keepT_d = nc.dram_tensor("keepT", [N, M], F16, kind="ExternalInput")
    wqT_d = nc.dram_tensor("wqT", [E, DLOC], F16, kind="ExternalInput")
    wkT_d = nc.dram_tensor("wkT", [E, DLOC], F16, kind="ExternalInput")
    wvT_d = nc.dram_tensor("wvT", [E, DLOC], F16, kind="ExternalInput")
    woT_d = nc.dram_tensor("woT", [DLOC, E], F16, kind="ExternalInput")
    out_d = nc.dram_tensor("out", [M, E], F32, kind="ExternalOutput")

    with tile.TileContext(nc) as tc, ExitStack() as ctx:
        const_pool = ctx.enter_context(tc.tile_pool(name="const", bufs=1))
        w_pool = ctx.enter_context(tc.tile_pool(name="weights", bufs=1))
        act_pool = ctx.enter_context(tc.tile_pool(name="acts", bufs=1))

        ones64 = const_pool.tile([1, 64], F16)
        nc.vector.memset(ones64[:], 1.0)

        wq_sb = w_pool.tile([128, ET * DLOC], F16, tag="wq")
        wk_sb = w_pool.tile([128, ET * DLOC], F16, tag="wk")
        wv_sb = w_pool.tile([128, ET * DLOC], F16, tag="wv")
        wo_sb = w_pool.tile([128, 2 * E], F16, tag="wo")
        for et in range(ET):
            sl = bass.ts(et, DLOC)
            nc.sync.dma_start(wq_sb[:, sl], wqT_d[bass.ts(et, 128), :])
            nc.sync.dma_start(wk_sb[:, sl], wkT_d[bass.ts(et, 128), :])
            nc.sync.dma_start(wv_sb[:, sl], wvT_d[bass.ts(et, 128), :])
        for kt in range(2):
            nc.sync.dma_start(wo_sb[:, bass.ts(kt, E)], woT_d[bass.ts(kt, 128), :])

        qw_sb = [act_pool.tile([128, M], F16, tag=f"qw{i}", name=f"qw{i}") for i in range(2)]
        kw_sb = [act_pool.tile([128, N], F16, tag=f"kw{i}", name=f"kw{i}") for i in range(2)]
        vw_sb = act_pool.tile([128, NT * VSTR], F16, tag="vw")
        ctx_sb = [act_pool.tile([128, M], F16, tag=f"ctx{i}", name=f"ctx{i}") for i in range(2)]
        nc.vector.memset(vw_sb[:], 1.0)  # pre-fill ones cols; data cols overwritten

        # ---- projections ----
        with (
            tc.tile_pool(name="xT", bufs=3) as xT_pool,
            tc.tile_pool(name="proj_ps", bufs=8, space="PSUM") as pps,
        ):
            def proj_qk(xT_dram, w_sb, out_t):
                ps = [pps.tile([128, 512], F32, tag="pp", name=f"pp{j2}") for j2 in range(8)]
                for et in range(ET):
                    xt = xT_pool.tile([128, M], F16, tag="xt")
                    nc.sync.dma_start(xt[:], xT_dram[bass.ts(et, 128), :])
                    for d2 in range(2):
                        for mc in range(4):
                            nc.tensor.matmul(
                                ps[d2 * 4 + mc][:],
                                w_sb[:, et * DLOC + d2 * 128 : et * DLOC + (d2 + 1) * 128],
                                xt[:, bass.ts(mc, 512)],
                                start=(et == 0), stop=(et == ET - 1),
                            )
                for d2 in range(2):
                    for mc in range(4):
                        nc.vector.tensor_copy(
                            out_t[d2][:, bass.ts(mc, 512)], ps[d2 * 4 + mc][:]
                        )

            proj_qk(qT_d, wq_sb, qw_sb)
            proj_qk(kT_d, wk_sb, kw_sb)

            # v projection: vw (n-tile, d_local) in 2 groups of 8 n-tiles
            for g in range(2):
                ps = [pps.tile([128, 512], F32, tag="pp", name=f"pp{j2}") for j2 in range(8)]
                for et in range(ET):
                    vt = xT_pool.tile([128, N], F16, tag="vt")
                    nc.sync.dma_start(vt[:], vT_d[bass.ts(et, 128), :])
                    for j in range(8):
                        nt = g * 8 + j
                        nc.tensor.matmul(
                            ps[j][:, 0:DLOC],
                            vt[:, bass.ts(nt, 128)],
                            wv_sb[:, bass.ts(et, DLOC)],
                            start=(et == 0), stop=(et == ET - 1),
                        )
                for j in range(8):
                    nt = g * 8 + j
                    for h in range(HL):
                        nc.vector.tensor_copy(
                            vw_sb[:, nt * VSTR + h * 65 : nt * VSTR + h * 65 + 64],
                            ps[j][:, bass.ts(h, 64)],
                        )

        # ---- attention ----
        with (
            tc.tile_pool(name="keep", bufs=1) as keep_pool,
            tc.tile_pool(name="s_ps", bufs=S_BUFS, space="PSUM") as s_ps,
            tc.tile_pool(name="c_ps", bufs=C_BUFS, space="PSUM") as c_ps,
            tc.tile_pool(name="attn", bufs=AU_BUFS) as attn_pool,
            tc.tile_pool(name="attnm", bufs=AM_BUFS) as attnm_pool,
            tc.tile_pool(name="eps", bufs=2) as eps_pool,
        ):
            for mh in range(2):  # m-halves of 1024
                moff = mh * 1024
                keep_sb = keep_pool.tile([128, NT * 1024], F16, tag="keep")
                for nt in range(NT):
                    nc.sync.dma_start(
                        keep_sb[:, bass.ts(nt, 1024)],
                        keepT_d[bass.ts(nt, 128), moff : moff + 1024],
                    )
                for h in range(HL):  # local heads, one at a time
                    hp, hl = divmod(h, 2)
                    pctx = c_ps.tile([65, 1024], F32, tag="pctx")
                    # software pipeline: ctx(nt - DEPTH) issues after
                    # scores/exp/mask(nt) so PE always has ready work
                    ams = {}
                    for step in range(NT + DEPTH):
                        if step < NT:
                            nt = step
                            ps = s_ps.tile([128, 1024], F32, tag="ps")
                            for mc2 in range(2):
                                nc.tensor.matmul(
                                    ps[:, bass.ts(mc2, 512)],
                                    kw_sb[hp][bass.ts(hl, 64), bass.ts(nt, 128)],
                                    qw_sb[hp][
                                        bass.ts(hl, 64),
                                        moff + mc2 * 512 : moff + (mc2 + 1) * 512,
                                    ],
                                    start=True, stop=True,
                                )
                            au = attn_pool.tile([128, 1024], F16, tag="au")
                            nc.scalar.activation(au[:], ps[:], EXP, scale=0.125)
                            am = attnm_pool.tile([128, 1024], F16, tag="am")
                            nc.vector.tensor_mul(
                                am[:], au[:], keep_sb[:, bass.ts(nt, 1024)]
                            )
                            ams[nt] = am
                        if step >= DEPTH:
                            nt = step - DEPTH
                            am = ams.pop(nt)
                            for mc2 in range(2):
                                nc.tensor.matmul(
                                    pctx[:, bass.ts(mc2, 512)],
                                    vw_sb[:, nt * VSTR + h * 65 : nt * VSTR + (h + 1) * 65],
                                    am[:, bass.ts(mc2, 512)],
                                    start=(nt == 0), stop=(nt == NT - 1),
                                )
                    # normalize: ctxT = pctx[0:64] * (1 / sums) ; sums = row 64
                    sums = eps_pool.tile([1, 1024], F16, tag="sums")
                    nc.vector.tensor_copy(sums[:], pctx[64:65, :])
                    prb = s_ps.tile([128, 1024], F32, tag="ps")
                    for mc2 in range(2):
                        nc.tensor.matmul(
                            prb[0:64, bass.ts(mc2, 512)],
                            ones64[:],
                            sums[:, bass.ts(mc2, 512)],
                            start=True, stop=True,
                        )
                    rbs = eps_pool.tile([64, 1024], F32, tag="rbs")
                    nc.vector.reciprocal_approx_fast(rbs[:], prb[0:64, :])
                    nc.vector.tensor_mul(
                        ctx_sb[hp][bass.ts(hl, 64), moff : moff + 1024],
                        pctx[0:64, :],
                        rbs[:],
                    )

        # ---- output projection (partial) ----
        with (
            tc.tile_pool(name="o_ps", bufs=3, space="PSUM") as o_ps,
            tc.tile_pool(name="o_sb", bufs=3) as o_sb_pool,
        ):
            for mt in range(M // 128):
                for ec in range(2):
                    po = o_ps.tile([128, 512], F32, tag="po")
                    for kt in range(2):
                        nc.tensor.matmul(
                            po[:],
                            ctx_sb[kt][:, bass.ts(mt, 128)],
                            wo_sb[:, kt * E + ec * 512 : kt * E + (ec + 1) * 512],
                            start=(kt == 0), stop=(kt == 1),
                        )
                    ob = o_sb_pool.tile([128, 512], F32, tag="ob")
                    nc.vector.tensor_copy(ob[:], po[:])
                    nc.sync.dma_start(
                        out_d[bass.ts(mt, 128), bass.ts(ec, 512)], ob[:]
                    )

    nc.finalize()
    return nc


_PROGRAM = None


def _get_program():
    global _PROGRAM
    if _PROGRAM is None:
        _PROGRAM = build_program()
    return _PROGRAM


def _make_in_maps(q, k, v, mask, Wq, Wk, Wv, Wo):
    q = np.asarray(q, dtype=np.float32)
    k = np.asarray(k, dtype=np.float32)
    v = np.asarray(v, dtype=np.float32)
    mask = np.asarray(mask)
    Wq = np.asarray(Wq, dtype=np.float32)
    Wk = np.asarray(Wk, dtype=np.float32)
    Wv = np.asarray(Wv, dtype=np.float32)
    Wo = np.asarray(Wo, dtype=np.float32)

    per_batch = {}
    for b in range(B):
        per_batch[b] = dict(
            qT=np.ascontiguousarray(q[b].T.astype(np.float16)),
            kT=np.ascontiguousarray(k[b].T.astype(np.float16)),
            vT=np.ascontiguousarray(v[b].T.astype(np.float16)),
            keepT=np.ascontiguousarray(
                np.logical_not(mask[b]).T.astype(np.float16)
            ),
        )

    in_maps = []
    for c in range(NCORES):
        b, hg = divmod(c, GROUPS)
        sl = slice(hg * DLOC, (hg + 1) * DLOC)
        in_maps.append(
            dict(
                per_batch[b],
                wqT=np.ascontiguousarray(Wq[sl].T.astype(np.float16)),
                wkT=np.ascontiguousarray(Wk[sl].T.astype(np.float16)),
                wvT=np.ascontiguousarray(Wv[sl].T.astype(np.float16)),
                woT=np.ascontiguousarray(Wo[:, sl].T.astype(np.float16)),
            )
        )
    return in_maps


def _run(in_maps, trace=False):
    nc = _get_program()
    return run_bass_kernel_spmd(
        nc, in_maps, list(range(NCORES)), trace=trace
    )


def _assemble(results):
    out = np.zeros((B, M, E), dtype=np.float32)
    for c in range(NCORES):
        b = c // GROUPS
        out[b] += results[c]["out"]
    return out


def kernel(q, k, v, mask, Wq, Wk, Wv, Wo):
    in_maps = _make_in_maps(q, k, v, mask, Wq, Wk, Wv, Wo)
    res = _run(in_maps, trace=False)
    return _assemble(res.results)


def run_profiled(q, k, v, mask, Wq, Wk, Wv, Wo):
    """Like kernel(), but traces execution; returns (out, BassKernelResults)."""
    in_maps = _make_in_maps(q, k, v, mask, Wq, Wk, Wv, Wo)
    res = _run(in_maps, trace=True)
    return _assemble(res.results), res


# revision 10
# speedup vs baseline: 1.1907x; 1.0979x over previous
"""Multi-head attention TRN2 Bass kernel (8 NeuronCores, tensor-parallel).

Sharding: Megatron-style TP over (batch x head-group). 8 cores = 2 batches x 4
head-groups of 4 heads each. Each core computes its heads' Q/K/V projections,
masked-softmax attention, and a partial output projection; the host sums the 4
partials per batch (the TP unshard).

Per-core kernel layout:
  qwT/kwT: (d_local=256, M=2048)   [d on partitions, 2 SBUF tensors of 128]
  vw:      (N=2048, d_local)       [n on partitions, +ones col per head]
  scoresT: (n-tile=128, m)  on PE -> PSUM (f32r matmuls, two heads packed
                                    into PE row-groups 0-1 / 2-3)
  exp:     ScalarE from PSUM, scale=1/8 folded in, -> fp16 SBUF
           (max-subtraction skipped: |scores|/8 <= ~6 so exp can't overflow)
  mask:    VectorE fp16 multiply by keep=(1-mask).T  (2x DVE mode)
  ctx:     PE fp16, lhsT=[vw_h | 1] (M=65) -> row 64 accumulates softmax sums
  norm:    approx-reciprocal + PE ones-broadcast, folded into PSUM->SBUF copy
  out:     partial (2048, 1024) = ctxT.T @ WoT_local
"""
import os
import sys

for p in ("/opt/trn_rl_repo",):
    if p not in sys.path:
        sys.path.insert(0, p)

from contextlib import ExitStack

import numpy as np

import concourse.bass as bass
import concourse.tile as tile
from concourse import bacc, mybir
from concourse.bass_utils import run_bass_kernel_spmd

F32 = mybir.dt.float32
F32R = mybir.dt.float32r
F16 = mybir.dt.float16
EXP = mybir.ActivationFunctionType.Exp

B, M, N, E = 2, 2048, 2048, 1024  # batch, q-len, k-len, d_model
H, DK = 16, 64                    # heads, head dim
NCORES = 8
GROUPS = 4                        # head groups (cores per batch)
DLOC = (H // GROUPS) * DK         # 256 per-core projection width
HL = H // GROUPS                  # 4 local heads
ET = E // 128                     # 8 e-tiles
NT = N // 128                     # 16 n-tiles
VSTR = HL * (DK + 1)              # 260: vw slot stride per n-tile

# tunables (env-overridable for experiments)
DEPTH = int(os.environ.get("K_DEPTH", "2"))
S_BUFS = int(os.environ.get("K_SBUFS", "3"))
C_BUFS = int(os.environ.get("K_CBUFS", "1"))
AU_BUFS = int(os.environ.get("K_AUBUFS", "6"))
AM_BUFS = int(os.environ.get("K_AMBUFS", "6"))


def build_program() -> bass.Bass:
    nc = bacc.Bacc()

    qT_d = nc.dram_tensor("qT", [E, M], F16, kind="ExternalInput")
    kT_d = nc.dram_tensor("kT", [E, N], F16, kind="ExternalInput")
    vT_d = nc.dram_tensor("vT", [E, N], F16, kind="ExternalInput")
    keepT_d = nc.dram_tensor("keepT", [N, M], F16, kind="ExternalInput")
    wqT_d = nc.dram_tensor("wqT", [E, DLOC], F16, kind="ExternalInput")
    wkT_d = nc.dram_tensor("wkT", [E, DLOC], F16, kind="ExternalInput")
    wvT_d = nc.dram_tensor("wvT", [E, DLOC], F16, kind="ExternalInput")
    woT_d = nc.dram_tensor("woT", [DLOC, E], F16, kind="ExternalInput")
    out_d = nc.dram_tensor("out", [M, E], F32, kind="ExternalOutput")

    with tile.TileContext(nc) as tc, ExitStack() as ctx:
        const_pool = ctx.enter_context(tc.tile_pool(name="const", bufs=1))
        w_pool = ctx.enter_context(tc.tile_pool(name="weights", bufs=1))
        act_pool = ctx.enter_context(tc.tile_pool(name="acts", bufs=1))

        ones64 = const_pool.tile([1, 64], F16)
        nc.vector.memset(ones64[:], 1.0)

        wq_sb = w_pool.tile([128, ET * DLOC], F16, tag="wq")
        wk_sb = w_pool.tile([128, ET * DLOC], F16, tag="wk")
        wv_sb = w_pool.tile([128, ET * DLOC], F16, tag="wv")
        wo_sb = w_pool.tile([128, 2 * E], F16, tag="wo")
        for et in range(ET):
            sl = bass.ts(et, DLOC)
            nc.sync.dma_start(wq_sb[:, sl], wqT_d[bass.ts(et, 128), :])
            nc.sync.dma_start(wk_sb[:, sl], wkT_d[bass.ts(et, 128), :])
            nc.sync.dma_start(wv_sb[:, sl], wvT_d[bass.ts(et, 128), :])
        for kt in range(2):
            nc.sync.dma_start(wo_sb[:, bass.ts(kt, E)], woT_d[bass.ts(kt, 128), :])

        qw_sb = [act_pool.tile([128, M], F16, tag=f"qw{i}", name=f"qw{i}") for i in range(2)]
        kw_sb = [act_pool.tile([128, N], F16, tag=f"kw{i}", name=f"kw{i}") for i in range(2)]
        vw_sb = act_pool.tile([128, NT * VSTR], F16, tag="vw")
        ctx_sb = [act_pool.tile([128, M], F16, tag=f"ctx{i}", name=f"ctx{i}") for i in range(2)]
        nc.vector.memset(vw_sb[:], 1.0)  # pre-fill ones cols; data cols overwritten

        # ---- projections ----
        with (
            tc.tile_pool(name="xT", bufs=3) as xT_pool,
            tc.tile_pool(name="proj_ps", bufs=8, space="PSUM") as pps,
        ):
            def proj_qk(xT_dram, w_sb, out_t):
                ps = [pps.tile([128, 512], F32, tag="pp", name=f"pp{j2}") for j2 in range(8)]
                for et in range(ET):
                    xt = xT_pool.tile([128, M], F16, tag="xt")
                    nc.sync.dma_start(xt[:], xT_dram[bass.ts(et, 128), :])
                    for d2 in range(2):
                        for mc in range(4):
                            nc.tensor.matmul(
                                ps[d2 * 4 + mc][:],
                                w_sb[:, et * DLOC + d2 * 128 : et * DLOC + (d2 + 1) * 128],
                                xt[:, bass.ts(mc, 512)],
                                start=(et == 0), stop=(et == ET - 1),
                            )
                for d2 in range(2):
                    for mc in range(4):
                        nc.vector.tensor_copy(
                            out_t[d2][:, bass.ts(mc, 512)], ps[d2 * 4 + mc][:]
                        )

            proj_qk(qT_d, wq_sb, qw_sb)
            proj_qk(kT_d, wk_sb, kw_sb)

            # v projection: vw (n-tile, d_local) in 2 groups of 8 n-tiles
            for g in range(2):
                ps = [pps.tile([128, 512], F32, tag="pp", name=f"pp{j2}") for j2 in range(8)]
                for et in range(ET):
                    vt = xT_pool.tile([128, N], F16, tag="vt")
                    nc.sync.dma_start(vt[:], vT_d[bass.ts(et, 128), :])
                    for j in range(8):
                        nt = g * 8 + j
                        nc.tensor.matmul(
                            ps[j][:, 0:DLOC],
                            vt[:, bass.ts(nt, 128)],
                            wv_sb[:, bass.ts(et, DLOC)],
                            start=(et == 0), stop=(et == ET - 1),
                        )
                for j in range(8):
                    nt = g * 8 + j
                    for h in range(HL):
                        nc.vector.tensor_copy(
                            vw_sb[:, nt * VSTR + h * 65 : nt * VSTR + h * 65 + 64],
                            ps[j][:, bass.ts(h, 64)],
                        )

        # ---- attention ----
        with (
            tc.tile_pool(name="keep", bufs=1) as keep_pool,
            tc.tile_pool(name="s_ps", bufs=S_BUFS, space="PSUM") as s_ps,
            tc.tile_pool(name="c_ps", bufs=C_BUFS, space="PSUM") as c_ps,
            tc.tile_pool(name="attn", bufs=AU_BUFS) as attn_pool,
            tc.tile_pool(name="attnm", bufs=AM_BUFS) as attnm_pool,
            tc.tile_pool(name="eps", bufs=2) as eps_pool,
        ):
            for mh in range(2):  # m-halves of 1024
                moff = mh * 1024
                keep_sb = keep_pool.tile([128, NT * 1024], F16, tag="keep")
                for nt in range(NT):
                    nc.sync.dma_start(
                        keep_sb[:, bass.ts(nt, 1024)],
                        keepT_d[bass.ts(nt, 128), moff : moff + 1024],
                    )
                for hp in range(2):  # head pairs; both heads in flight
                    pctx = [
                        c_ps.tile([65, 1024], F32, tag="pctx", name=f"pctx{j2}")
                        for j2 in range(2)
                    ]
                    # software pipeline: ctx(nt - DEPTH) issues after
                    # scores/exp/mask(nt); the pair's two scores matmuls run
                    # CONCURRENTLY in PE row-groups 0-1 / 2-3 (full array ->
                    # HAM warms to 2.4 GHz)
                    ams = {}
                    for step in range(NT + DEPTH):
                        if step < NT:
                            nt = step
                            pss = [
                                s_ps.tile([128, 1024], F32, tag="ps", name=f"ps{j2}")
                                for j2 in range(2)
                            ]
                            for hl in range(2):
                                for mc2 in range(2):
                                    nc.tensor.matmul(
                                        pss[hl][:, bass.ts(mc2, 512)],
                                        kw_sb[hp][bass.ts(hl, 64), bass.ts(nt, 128)],
                                        qw_sb[hp][
                                            bass.ts(hl, 64),
                                            moff + mc2 * 512 : moff + (mc2 + 1) * 512,
                                        ],
                                        start=True, stop=True,
                                    )
                            for hl in range(2):
                                au = attn_pool.tile([128, 1024], F16, tag="au")
                                nc.scalar.activation(au[:], pss[hl][:], EXP, scale=0.125)
                                am = attnm_pool.tile([128, 1024], F16, tag="am")
                                nc.vector.tensor_mul(
                                    am[:], au[:], keep_sb[:, bass.ts(nt, 1024)]
                                )
                                ams[(nt, hl)] = am
                        if step >= DEPTH:
                            nt = step - DEPTH
                            for hl in range(2):
                                h = hp * 2 + hl
                                am = ams.pop((nt, hl))
                                for mc2 in range(2):
                                    nc.tensor.matmul(
                                        pctx[hl][:, bass.ts(mc2, 512)],
                                        vw_sb[:, nt * VSTR + h * 65 : nt * VSTR + (h + 1) * 65],
                                        am[:, bass.ts(mc2, 512)],
                                        start=(nt == 0), stop=(nt == NT - 1),
                                    )
                    # normalize: ctxT = pctx[0:64] * (1 / sums) ; sums = row 64
                    for hl in range(2):
                        sums = eps_pool.tile([1, 1024], F16, tag="sums")
                        nc.vector.tensor_copy(sums[:], pctx[hl][64:65, :])
                        prb = s_ps.tile([128, 1024], F32, tag="ps")
                        for mc2 in range(2):
                            nc.tensor.matmul(
                                prb[0:64, bass.ts(mc2, 512)],
                                ones64[:],
                                sums[:, bass.ts(mc2, 512)],
                                start=True, stop=True,
                            )
                        rbs = eps_pool.tile([64, 1024], F32, tag="rbs")
                        nc.vector.reciprocal_approx_fast(rbs[:], prb[0:64, :])
                        nc.vector.tensor_mul(
                            ctx_sb[hp][bass.ts(hl, 64), moff : moff + 1024],
                            pctx[hl][0:64, :],
                            rbs[:],
                        )

        # ---- output projection (partial) ----
        with (
            tc.tile_pool(name="o_ps", bufs=3, space="PSUM") as o_ps,
            tc.tile_pool(name="o_sb", bufs=3) as o_sb_pool,
        ):
            for mt in range(M // 128):
                for ec in range(2):
                    po = o_ps.tile([128, 512], F32, tag="po")
                    for kt in range(2):
                        nc.tensor.matmul(
                            po[:],
                            ctx_sb[kt][:, bass.ts(mt, 128)],
                            wo_sb[:, kt * E + ec * 512 : kt * E + (ec + 1) * 512],
                            start=(kt == 0), stop=(kt == 1),
                        )
                    ob = o_sb_pool.tile([128, 512], F32, tag="ob")
                    nc.vector.tensor_copy(ob[:], po[:])
                    nc.sync.dma_start(
                        out_d[bass.ts(mt, 128), bass.ts(ec, 512)], ob[:]
                    )

    nc.finalize()
    return nc


_PROGRAM = None


def _get_program():
    global _PROGRAM
    if _PROGRAM is None:
        _PROGRAM = build_program()
    return _PROGRAM


def _make_in_maps(q, k, v, mask, Wq, Wk, Wv, Wo):
    q = np.asarray(q, dtype=np.float32)
    k = np.asarray(k, dtype=np.float32)
    v = np.asarray(v, dtype=np.float32)
    mask = np.asarray(mask)
    Wq = np.asarray(Wq, dtype=np.float32)
    Wk = np.asarray(Wk, dtype=np.float32)
    Wv = np.asarray(Wv, dtype=np.float32)
    Wo = np.asarray(Wo, dtype=np.float32)

    per_batch = {}
    for b in range(B):
        per_batch[b] = dict(
            qT=np.ascontiguousarray(q[b].T.astype(np.float16)),
            kT=np.ascontiguousarray(k[b].T.astype(np.float16)),
            vT=np.ascontiguousarray(v[b].T.astype(np.float16)),
            keepT=np.ascontiguousarray(
                np.logical_not(mask[b]).T.astype(np.float16)
            ),
        )

    in_maps = []
    for c in range(NCORES):
        b, hg = divmod(c, GROUPS)
        sl = slice(hg * DLOC, (hg + 1) * DLOC)
        in_maps.append(
            dict(
                per_batch[b],
                wqT=np.ascontiguousarray(Wq[sl].T.astype(np.float16)),
                wkT=np.ascontiguousarray(Wk[sl].T.astype(np.float16)),
                wvT=np.ascontiguousarray(Wv[sl].T.astype(np.float16)),
                woT=np.ascontiguousarray(Wo[:, sl].T.astype(np.float16)),
            )
        )
    return in_maps


def _run(in_maps, trace=False):
    nc = _get_program()
    return run_bass_kernel_spmd(
        nc, in_maps, list(range(NCORES)), trace=trace
    )


def _assemble(results):
    out = np.zeros((B, M, E), dtype=np.float32)
    for c in range(NCORES):
        b = c // GROUPS
        out[b] += results[c]["out"]
    return out


def kernel(q, k, v, mask, Wq, Wk, Wv, Wo):
    in_maps = _make_in_maps(q, k, v, mask, Wq, Wk, Wv, Wo)
    res = _run(in_maps, trace=False)
    return _assemble(res.results)


def run_profiled(q, k, v, mask, Wq, Wk, Wv, Wo):
    """Like kernel(), but traces execution; returns (out, BassKernelResults)."""
    in_maps = _make_in_maps(q, k, v, mask, Wq, Wk, Wv, Wo)
    res = _run(in_maps, trace=True)
    return _assemble(res.results), res


# revision 11
# speedup vs baseline: 1.3867x; 1.1646x over previous
"""Multi-head attention TRN2 Bass kernel (8 NeuronCores, tensor-parallel).

Sharding: Megatron-style TP over (batch x head-group). 8 cores = 2 batches x 4
head-groups of 4 heads each. Each core computes its heads' Q/K/V projections,
masked-softmax attention, and a partial output projection; the host sums the 4
partials per batch (the TP unshard).

Per-core kernel layout:
  qwT/kwT: (d_local=256, M=2048)   [d on partitions, 2 SBUF tensors of 128]
  vw:      (N=2048, d_local)       [n on partitions, +ones col per head]
  scoresT: (n-tile=128, m)  on PE -> PSUM (f32r matmuls, two heads packed
                                    into PE row-groups 0-1 / 2-3)
  exp:     ScalarE from PSUM, scale=1/8 folded in, -> fp16 SBUF
           (max-subtraction skipped: |scores|/8 <= ~6 so exp can't overflow)
  mask:    VectorE fp16 multiply by keep=(1-mask).T  (2x DVE mode)
  ctx:     PE fp16, lhsT=[vw_h | 1] (M=65) -> row 64 accumulates softmax sums
  norm:    approx-reciprocal + PE ones-broadcast, folded into PSUM->SBUF copy
  out:     partial (2048, 1024) = ctxT.T @ WoT_local
"""
import os
import sys

for p in ("/opt/trn_rl_repo",):
    if p not in sys.path:
        sys.path.insert(0, p)

from contextlib import ExitStack

import numpy as np

import concourse.bass as bass
import concourse.tile as tile
from concourse import bacc, mybir
from concourse.bass_utils import run_bass_kernel_spmd

F32 = mybir.dt.float32
F32R = mybir.dt.float32r
F16 = mybir.dt.float16
EXP = mybir.ActivationFunctionType.Exp

B, M, N, E = 2, 2048, 2048, 1024  # batch, q-len, k-len, d_model
H, DK = 16, 64                    # heads, head dim
NCORES = 8
GROUPS = 4                        # head groups (cores per batch)
DLOC = (H // GROUPS) * DK         # 256 per-core projection width
HL = H // GROUPS                  # 4 local heads
ET = E // 128                     # 8 e-tiles
NT = N // 128                     # 16 n-tiles
VSTR = HL * (DK + 1)              # 260: vw slot stride per n-tile

# tunables (env-overridable for experiments)
DEPTH = int(os.environ.get("K_DEPTH", "2"))
S_BUFS = int(os.environ.get("K_SBUFS", "3"))
C_BUFS = int(os.environ.get("K_CBUFS", "1"))
AU_BUFS = int(os.environ.get("K_AUBUFS", "6"))
AM_BUFS = int(os.environ.get("K_AMBUFS", "6"))


def build_program() -> bass.Bass:
    nc = bacc.Bacc()

    qT_d = nc.dram_tensor("qT", [E, M], F16, kind="ExternalInput")
    kT_d = nc.dram_tensor("kT", [E, N], F16, kind="ExternalInput")
    vT_d = nc.dram_tensor("vT", [E, N], F16, kind="ExternalInput")
    keepT_d = nc.dram_tensor("keepT", [N, M], F16, kind="ExternalInput")
    wqT_d = nc.dram_tensor("wqT", [E, DLOC], F16, kind="ExternalInput")
    wkT_d = nc.dram_tensor("wkT", [E, DLOC], F16, kind="ExternalInput")
    wvT_d = nc.dram_tensor("wvT", [E, DLOC], F16, kind="ExternalInput")
    woT_d = nc.dram_tensor("woT", [DLOC, E], F16, kind="ExternalInput")
    out_d = nc.dram_tensor("out", [M, E], F32, kind="ExternalOutput")

    with tile.TileContext(nc) as tc, ExitStack() as ctx:
        const_pool = ctx.enter_context(tc.tile_pool(name="const", bufs=1))
        w_pool = ctx.enter_context(tc.tile_pool(name="weights", bufs=1))
        act_pool = ctx.enter_context(tc.tile_pool(name="acts", bufs=1))

        ones64 = const_pool.tile([1, 64], F16)
        nc.vector.memset(ones64[:], 1.0)

        wq_sb = w_pool.tile([128, ET * DLOC], F16, tag="wq")
        wk_sb = w_pool.tile([128, ET * DLOC], F16, tag="wk")
        wv_sb = w_pool.tile([128, ET * DLOC], F16, tag="wv")
        wo_sb = w_pool.tile([128, 2 * E], F16, tag="wo")
        for et in range(ET):
            sl = bass.ts(et, DLOC)
            nc.sync.dma_start(wq_sb[:, sl], wqT_d[bass.ts(et, 128), :])
            nc.sync.dma_start(wk_sb[:, sl], wkT_d[bass.ts(et, 128), :])
            nc.sync.dma_start(wv_sb[:, sl], wvT_d[bass.ts(et, 128), :])
        for kt in range(2):
            nc.sync.dma_start(wo_sb[:, bass.ts(kt, E)], woT_d[bass.ts(kt, 128), :])

        # qwz[hp][hl]: full-128-partition qw with the OTHER head's 64 rows
        # zeroed -> scores matmuls use K=128 (full PE rows; keeps HAM warm)
        # with a single shared kw lhsT per (hp, nt).
        qwz = [
            [act_pool.tile([128, M], F16, tag=f"qwz{i}{j}", name=f"qwz{i}{j}")
             for j in range(2)]
            for i in range(2)
        ]
        for i in range(2):
            nc.vector.memset(qwz[i][0][bass.ts(1, 64), :], 0.0)
            nc.vector.memset(qwz[i][1][bass.ts(0, 64), :], 0.0)
        kw_sb = [act_pool.tile([128, N], F16, tag=f"kw{i}", name=f"kw{i}") for i in range(2)]
        vw_sb = act_pool.tile([128, NT * VSTR], F16, tag="vw")
        ctx_sb = [act_pool.tile([128, M], F16, tag=f"ctx{i}", name=f"ctx{i}") for i in range(2)]
        nc.vector.memset(vw_sb[:], 1.0)  # pre-fill ones cols; data cols overwritten

        # ---- projections ----
        with (
            tc.tile_pool(name="xT", bufs=3) as xT_pool,
            tc.tile_pool(name="proj_ps", bufs=8, space="PSUM") as pps,
        ):
            def proj_qk(xT_dram, w_sb, writer):
                ps = [pps.tile([128, 512], F32, tag="pp", name=f"pp{j2}") for j2 in range(8)]
                for et in range(ET):
                    xt = xT_pool.tile([128, M], F16, tag="xt")
                    nc.sync.dma_start(xt[:], xT_dram[bass.ts(et, 128), :])
                    for d2 in range(2):
                        for mc in range(4):
                            nc.tensor.matmul(
                                ps[d2 * 4 + mc][:],
                                w_sb[:, et * DLOC + d2 * 128 : et * DLOC + (d2 + 1) * 128],
                                xt[:, bass.ts(mc, 512)],
                                start=(et == 0), stop=(et == ET - 1),
                            )
                for d2 in range(2):
                    for mc in range(4):
                        writer(d2, mc, ps[d2 * 4 + mc])

            def q_writer(d2, mc, ps):
                for hl in range(2):
                    nc.vector.tensor_copy(
                        qwz[d2][hl][bass.ts(hl, 64), bass.ts(mc, 512)],
                        ps[bass.ts(hl, 64), :],
                    )

            def k_writer(d2, mc, ps):
                nc.vector.tensor_copy(kw_sb[d2][:, bass.ts(mc, 512)], ps[:])

            proj_qk(qT_d, wq_sb, q_writer)
            proj_qk(kT_d, wk_sb, k_writer)

            # v projection: vw (n-tile, d_local) in 2 groups of 8 n-tiles
            for g in range(2):
                ps = [pps.tile([128, 512], F32, tag="pp", name=f"pp{j2}") for j2 in range(8)]
                for et in range(ET):
                    vt = xT_pool.tile([128, N], F16, tag="vt")
                    nc.sync.dma_start(vt[:], vT_d[bass.ts(et, 128), :])
                    for j in range(8):
                        nt = g * 8 + j
                        nc.tensor.matmul(
                            ps[j][:, 0:DLOC],
                            vt[:, bass.ts(nt, 128)],
                            wv_sb[:, bass.ts(et, DLOC)],
                            start=(et == 0), stop=(et == ET - 1),
                        )
                for j in range(8):
                    nt = g * 8 + j
                    for h in range(HL):
                        nc.vector.tensor_copy(
                            vw_sb[:, nt * VSTR + h * 65 : nt * VSTR + h * 65 + 64],
                            ps[j][:, bass.ts(h, 64)],
                        )

        # ---- attention ----
        with (
            tc.tile_pool(name="keep", bufs=1) as keep_pool,
            tc.tile_pool(name="s_ps", bufs=S_BUFS, space="PSUM") as s_ps,
            tc.tile_pool(name="c_ps", bufs=C_BUFS, space="PSUM") as c_ps,
            tc.tile_pool(name="attn", bufs=AU_BUFS) as attn_pool,
            tc.tile_pool(name="attnm", bufs=AM_BUFS) as attnm_pool,
            tc.tile_pool(name="eps", bufs=2) as eps_pool,
        ):
            for mh in range(2):  # m-halves of 1024
                moff = mh * 1024
                keep_sb = keep_pool.tile([128, NT * 1024], F16, tag="keep")
                for nt in range(NT):
                    nc.sync.dma_start(
                        keep_sb[:, bass.ts(nt, 1024)],
                        keepT_d[bass.ts(nt, 128), moff : moff + 1024],
                    )
                for hp in range(2):  # head pairs; both heads in flight
                    pctx = [
                        c_ps.tile([65, 1024], F32, tag="pctx", name=f"pctx{j2}")
                        for j2 in range(2)
                    ]
                    # software pipeline: ctx(nt - DEPTH) issues after
                    # scores/exp/mask(nt); the pair's two scores matmuls run
                    # CONCURRENTLY in PE row-groups 0-1 / 2-3 (full array ->
                    # HAM warms to 2.4 GHz)
                    ams = {}
                    for step in range(NT + DEPTH):
                        if step < NT:
                            nt = step
                            pss = [
                                s_ps.tile([128, 1024], F32, tag="ps", name=f"ps{j2}")
                                for j2 in range(2)
                            ]
                            for hl in range(2):
                                for mc2 in range(2):
                                    nc.tensor.matmul(
                                        pss[hl][:, bass.ts(mc2, 512)],
                                        kw_sb[hp][:, bass.ts(nt, 128)],
                                        qwz[hp][hl][
                                            :,
                                            moff + mc2 * 512 : moff + (mc2 + 1) * 512,
                                        ],
                                        start=True, stop=True,
                                    )
                            for hl in range(2):
                                au = attn_pool.tile([128, 1024], F16, tag="au")
                                nc.scalar.activation(au[:], pss[hl][:], EXP, scale=0.125)
                                am = attnm_pool.tile([128, 1024], F16, tag="am")
                                nc.vector.tensor_mul(
                                    am[:], au[:], keep_sb[:, bass.ts(nt, 1024)]
                                )
                                ams[(nt, hl)] = am
                        if step >= DEPTH:
                            nt = step - DEPTH
                            for hl in range(2):
                                h = hp * 2 + hl
                                am = ams.pop((nt, hl))
                                for mc2 in range(2):
                                    nc.tensor.matmul(
                                        pctx[hl][:, bass.ts(mc2, 512)],
                                        vw_sb[:, nt * VSTR + h * 65 : nt * VSTR + (h + 1) * 65],
                                        am[:, bass.ts(mc2, 512)],
                                        start=(nt == 0), stop=(nt == NT - 1),
                                    )
                    # normalize: ctxT = pctx[0:64] * (1 / sums) ; sums = row 64
                    for hl in range(2):
                        sums = eps_pool.tile([1, 1024], F16, tag="sums")
                        nc.vector.tensor_copy(sums[:], pctx[hl][64:65, :])
                        prb = s_ps.tile([128, 1024], F32, tag="ps")
                        for mc2 in range(2):
                            nc.tensor.matmul(
                                prb[0:64, bass.ts(mc2, 512)],
                                ones64[:],
                                sums[:, bass.ts(mc2, 512)],
                                start=True, stop=True,
                            )
                        rbs = eps_pool.tile([64, 1024], F32, tag="rbs")
                        nc.vector.reciprocal_approx_fast(rbs[:], prb[0:64, :])
                        nc.vector.tensor_mul(
                            ctx_sb[hp][bass.ts(hl, 64), moff : moff + 1024],
                            pctx[hl][0:64, :],
                            rbs[:],
                        )

        # ---- output projection (partial) ----
        with (
            tc.tile_pool(name="o_ps", bufs=3, space="PSUM") as o_ps,
            tc.tile_pool(name="o_sb", bufs=3) as o_sb_pool,
        ):
            for mt in range(M // 128):
                for ec in range(2):
                    po = o_ps.tile([128, 512], F32, tag="po")
                    for kt in range(2):
                        nc.tensor.matmul(
                            po[:],
                            ctx_sb[kt][:, bass.ts(mt, 128)],
                            wo_sb[:, kt * E + ec * 512 : kt * E + (ec + 1) * 512],
                            start=(kt == 0), stop=(kt == 1),
                        )
                    ob = o_sb_pool.tile([128, 512], F32, tag="ob")
                    nc.vector.tensor_copy(ob[:], po[:])
                    nc.sync.dma_start(
                        out_d[bass.ts(mt, 128), bass.ts(ec, 512)], ob[:]
                    )

    nc.finalize()
    return nc


_PROGRAM = None


def _get_program():
    global _PROGRAM
    if _PROGRAM is None:
        _PROGRAM = build_program()
    return _PROGRAM


def _make_in_maps(q, k, v, mask, Wq, Wk, Wv, Wo):
    q = np.asarray(q, dtype=np.float32)
    k = np.asarray(k, dtype=np.float32)
    v = np.asarray(v, dtype=np.float32)
    mask = np.asarray(mask)
    Wq = np.asarray(Wq, dtype=np.float32)
    Wk = np.asarray(Wk, dtype=np.float32)
    Wv = np.asarray(Wv, dtype=np.float32)
    Wo = np.asarray(Wo, dtype=np.float32)

    per_batch = {}
    for b in range(B):
        per_batch[b] = dict(
            qT=np.ascontiguousarray(q[b].T.astype(np.float16)),
            kT=np.ascontiguousarray(k[b].T.astype(np.float16)),
            vT=np.ascontiguousarray(v[b].T.astype(np.float16)),
            keepT=np.ascontiguousarray(
                np.logical_not(mask[b]).T.astype(np.float16)
            ),
        )

    in_maps = []
    for c in range(NCORES):
        b, hg = divmod(c, GROUPS)
        sl = slice(hg * DLOC, (hg + 1) * DLOC)
        in_maps.append(
            dict(
                per_batch[b],
                wqT=np.ascontiguousarray(Wq[sl].T.astype(np.float16)),
                wkT=np.ascontiguousarray(Wk[sl].T.astype(np.float16)),
                wvT=np.ascontiguousarray(Wv[sl].T.astype(np.float16)),
                woT=np.ascontiguousarray(Wo[:, sl].T.astype(np.float16)),
            )
        )
    return in_maps


def _run(in_maps, trace=False):
    nc = _get_program()
    return run_bass_kernel_spmd(
        nc, in_maps, list(range(NCORES)), trace=trace
    )


def _assemble(results):
    out = np.zeros((B, M, E), dtype=np.float32)
    for c in range(NCORES):
        b = c // GROUPS
        out[b] += results[c]["out"]
    return out


def kernel(q, k, v, mask, Wq, Wk, Wv, Wo):
    in_maps = _make_in_maps(q, k, v, mask, Wq, Wk, Wv, Wo)
    res = _run(in_maps, trace=False)
    return _assemble(res.results)


def run_profiled(q, k, v, mask, Wq, Wk, Wv, Wo):
    """Like kernel(), but traces execution; returns (out, BassKernelResults)."""
    in_maps = _make_in_maps(q, k, v, mask, Wq, Wk, Wv, Wo)
    res = _run(in_maps, trace=True)
    return _assemble(res.results), res


# revision 12
# speedup vs baseline: 1.5129x; 1.0911x over previous
"""Multi-head attention TRN2 Bass kernel (8 NeuronCores, tensor-parallel).

Sharding: Megatron-style TP over (batch x head-group). 8 cores = 2 batches x 4
head-groups of 4 heads each. Each core computes its heads' Q/K/V projections,
masked-softmax attention, and a partial output projection; the host sums the 4
partials per batch (the TP unshard).

Per-core kernel layout:
  qwT/kwT: (d_local=256, M=2048)   [d on partitions, 2 SBUF tensors of 128]
  vw:      (N=2048, d_local)       [n on partitions, +ones col per head]
  scoresT: (n-tile=128, m)  on PE -> PSUM (f32r matmuls, two heads packed
                                    into PE row-groups 0-1 / 2-3)
  exp:     ScalarE from PSUM, scale=1/8 folded in, -> fp16 SBUF
           (max-subtraction skipped: |scores|/8 <= ~6 so exp can't overflow)
  mask:    VectorE fp16 multiply by keep=(1-mask).T  (2x DVE mode)
  ctx:     PE fp16, lhsT=[vw_h | 1] (M=65) -> row 64 accumulates softmax sums
  norm:    approx-reciprocal + PE ones-broadcast, folded into PSUM->SBUF copy
  out:     partial (2048, 1024) = ctxT.T @ WoT_local
"""
import os
import sys

for p in ("/opt/trn_rl_repo",):
    if p not in sys.path:
        sys.path.insert(0, p)

from contextlib import ExitStack

import numpy as np

import concourse.bass as bass
import concourse.tile as tile
from concourse import bacc, mybir
from concourse.bass_utils import run_bass_kernel_spmd

F32 = mybir.dt.float32
F32R = mybir.dt.float32r
F16 = mybir.dt.float16
EXP = mybir.ActivationFunctionType.Exp

B, M, N, E = 2, 2048, 2048, 1024  # batch, q-len, k-len, d_model
H, DK = 16, 64                    # heads, head dim
NCORES = 8
GROUPS = 4                        # head groups (cores per batch)
DLOC = (H // GROUPS) * DK         # 256 per-core projection width
HL = H // GROUPS                  # 4 local heads
ET = E // 128                     # 8 e-tiles
NT = N // 128                     # 16 n-tiles
VSTR = HL * (DK + 1)              # 260: vw slot stride per n-tile

# tunables (env-overridable for experiments)
DEPTH = int(os.environ.get("K_DEPTH", "2"))
S_BUFS = int(os.environ.get("K_SBUFS", "3"))
C_BUFS = int(os.environ.get("K_CBUFS", "1"))
AU_BUFS = int(os.environ.get("K_AUBUFS", "6"))
AM_BUFS = int(os.environ.get("K_AMBUFS", "6"))


def build_program() -> bass.Bass:
    nc = bacc.Bacc()

    qT_d = nc.dram_tensor("qT", [E, M], F16, kind="ExternalInput")
    kT_d = nc.dram_tensor("kT", [E, N], F16, kind="ExternalInput")
    vT_d = nc.dram_tensor("vT", [E, N], F16, kind="ExternalInput")
    keepT_d = nc.dram_tensor("keepT", [N, M], F16, kind="ExternalInput")
    wqT_d = nc.dram_tensor("wqT", [E, DLOC], F16, kind="ExternalInput")
    wkT_d = nc.dram_tensor("wkT", [E, DLOC], F16, kind="ExternalInput")
    wvT_d = nc.dram_tensor("wvT", [E, DLOC], F16, kind="ExternalInput")
    woT_d = nc.dram_tensor("woT", [DLOC, E], F16, kind="ExternalInput")
    out_d = nc.dram_tensor("out", [M, E], F32, kind="ExternalOutput")

    with tile.TileContext(nc) as tc, ExitStack() as ctx:
        const_pool = ctx.enter_context(tc.tile_pool(name="const", bufs=1))
        w_pool = ctx.enter_context(tc.tile_pool(name="weights", bufs=1))
        act_pool = ctx.enter_context(tc.tile_pool(name="acts", bufs=1))

        ones64 = const_pool.tile([1, 64], F16)
        nc.vector.memset(ones64[:], 1.0)

        wq_sb = w_pool.tile([128, ET * DLOC], F16, tag="wq")
        wk_sb = w_pool.tile([128, ET * DLOC], F16, tag="wk")
        wv_sb = w_pool.tile([128, ET * DLOC], F16, tag="wv")
        wo_sb = w_pool.tile([128, 2 * E], F16, tag="wo")

        # qwz[hp][hl]: full-128-partition qw with the OTHER head's 64 rows
        # zeroed -> scores matmuls use K=128 (full PE rows; keeps HAM warm)
        # with a single shared kw lhsT per (hp, nt).
        qwz = [
            [act_pool.tile([128, M], F16, tag=f"qwz{i}{j}", name=f"qwz{i}{j}")
             for j in range(2)]
            for i in range(2)
        ]
        for i in range(2):
            nc.vector.memset(qwz[i][0][bass.ts(1, 64), :], 0.0)
            nc.vector.memset(qwz[i][1][bass.ts(0, 64), :], 0.0)
        kw_sb = [act_pool.tile([128, N], F16, tag=f"kw{i}", name=f"kw{i}") for i in range(2)]
        vw_sb = act_pool.tile([128, NT * VSTR], F16, tag="vw")
        ctx_sb = [act_pool.tile([128, M], F16, tag=f"ctx{i}", name=f"ctx{i}") for i in range(2)]
        nc.vector.memset(vw_sb[:], 1.0)  # pre-fill ones cols; data cols overwritten

        # ---- projections ----
        with (
            tc.tile_pool(name="xT", bufs=3) as xT_pool,
            tc.tile_pool(name="proj_ps", bufs=8, space="PSUM") as pps,
        ):
            def proj_qk(xT_dram, w_dram, w_sb, writer):
                ps = [pps.tile([128, 512], F32, tag="pp", name=f"pp{j2}") for j2 in range(8)]
                for et in range(ET):
                    nc.sync.dma_start(
                        w_sb[:, bass.ts(et, DLOC)], w_dram[bass.ts(et, 128), :]
                    )
                    xt = xT_pool.tile([128, M], F16, tag="xt")
                    nc.sync.dma_start(xt[:], xT_dram[bass.ts(et, 128), :])
                    for d2 in range(2):
                        for mc in range(4):
                            nc.tensor.matmul(
                                ps[d2 * 4 + mc][:],
                                w_sb[:, et * DLOC + d2 * 128 : et * DLOC + (d2 + 1) * 128],
                                xt[:, bass.ts(mc, 512)],
                                start=(et == 0), stop=(et == ET - 1),
                            )
                for d2 in range(2):
                    for mc in range(4):
                        writer(d2, mc, ps[d2 * 4 + mc])

            def q_writer(d2, mc, ps):
                for hl in range(2):
                    nc.vector.tensor_copy(
                        qwz[d2][hl][bass.ts(hl, 64), bass.ts(mc, 512)],
                        ps[bass.ts(hl, 64), :],
                    )

            def k_writer(d2, mc, ps):
                nc.vector.tensor_copy(kw_sb[d2][:, bass.ts(mc, 512)], ps[:])

            proj_qk(qT_d, wqT_d, wq_sb, q_writer)
            proj_qk(kT_d, wkT_d, wk_sb, k_writer)

            # v projection: vw (n-tile, d_local) in 2 groups of 8 n-tiles
            for g in range(2):
                ps = [pps.tile([128, 512], F32, tag="pp", name=f"pp{j2}") for j2 in range(8)]
                for et in range(ET):
                    if g == 0:
                        nc.sync.dma_start(
                            wv_sb[:, bass.ts(et, DLOC)], wvT_d[bass.ts(et, 128), :]
                        )
                    vt = xT_pool.tile([128, N], F16, tag="vt")
                    nc.sync.dma_start(vt[:], vT_d[bass.ts(et, 128), :])
                    for j in range(8):
                        nt = g * 8 + j
                        nc.tensor.matmul(
                            ps[j][:, 0:DLOC],
                            vt[:, bass.ts(nt, 128)],
                            wv_sb[:, bass.ts(et, DLOC)],
                            start=(et == 0), stop=(et == ET - 1),
                        )
                for j in range(8):
                    nt = g * 8 + j
                    for h in range(HL):
                        nc.vector.tensor_copy(
                            vw_sb[:, nt * VSTR + h * 65 : nt * VSTR + h * 65 + 64],
                            ps[j][:, bass.ts(h, 64)],
                        )

        # ---- attention ----
        with (
            tc.tile_pool(name="keep", bufs=1) as keep_pool,
            tc.tile_pool(name="s_ps", bufs=S_BUFS, space="PSUM") as s_ps,
            tc.tile_pool(name="c_ps", bufs=C_BUFS, space="PSUM") as c_ps,
            tc.tile_pool(name="attn", bufs=AU_BUFS) as attn_pool,
            tc.tile_pool(name="attnm", bufs=AM_BUFS) as attnm_pool,
            tc.tile_pool(name="eps", bufs=2) as eps_pool,
        ):
            for mh in range(2):  # m-halves of 1024
                moff = mh * 1024
                keep_sb = keep_pool.tile([128, NT * 1024], F16, tag="keep")
                for nt in range(NT):
                    nc.sync.dma_start(
                        keep_sb[:, bass.ts(nt, 1024)],
                        keepT_d[bass.ts(nt, 128), moff : moff + 1024],
                    )
                for hp in range(2):  # head pairs; both heads in flight
                    pctx = [
                        c_ps.tile([65, 1024], F32, tag="pctx", name=f"pctx{j2}")
                        for j2 in range(2)
                    ]
                    # software pipeline: ctx(nt - DEPTH) issues after
                    # scores/exp/mask(nt); the pair's two scores matmuls run
                    # CONCURRENTLY in PE row-groups 0-1 / 2-3 (full array ->
                    # HAM warms to 2.4 GHz)
                    ams = {}
                    for step in range(NT + DEPTH):
                        if step < NT:
                            nt = step
                            pss = [
                                s_ps.tile([128, 1024], F32, tag="ps", name=f"ps{j2}")
                                for j2 in range(2)
                            ]
                            for hl in range(2):
                                for mc2 in range(2):
                                    nc.tensor.matmul(
                                        pss[hl][:, bass.ts(mc2, 512)],
                                        kw_sb[hp][:, bass.ts(nt, 128)],
                                        qwz[hp][hl][
                                            :,
                                            moff + mc2 * 512 : moff + (mc2 + 1) * 512,
                                        ],
                                        start=True, stop=True,
                                    )
                            for hl in range(2):
                                au = attn_pool.tile([128, 1024], F16, tag="au")
                                nc.scalar.activation(au[:], pss[hl][:], EXP, scale=0.125)
                                am = attnm_pool.tile([128, 1024], F16, tag="am")
                                nc.vector.tensor_mul(
                                    am[:], au[:], keep_sb[:, bass.ts(nt, 1024)]
                                )
                                ams[(nt, hl)] = am
                        if step >= DEPTH:
                            nt = step - DEPTH
                            for hl in range(2):
                                h = hp * 2 + hl
                                am = ams.pop((nt, hl))
                                for mc2 in range(2):
                                    nc.tensor.matmul(
                                        pctx[hl][:, bass.ts(mc2, 512)],
                                        vw_sb[:, nt * VSTR + h * 65 : nt * VSTR + (h + 1) * 65],
                                        am[:, bass.ts(mc2, 512)],
                                        start=(nt == 0), stop=(nt == NT - 1),
                                    )
                    # normalize: ctxT = pctx[0:64] * (1 / sums) ; sums = row 64.
                    # First copy pctx to SBUF so the PSUM banks free up
                    # immediately; the 4-hop normalize chain then runs off
                    # the critical path.
                    for hl in range(2):
                        csb = eps_pool.tile([65, 1024], F32, tag="csb")
                        nc.vector.tensor_copy(csb[:], pctx[hl][:])
                        sums = eps_pool.tile([1, 1024], F16, tag="sums")
                        nc.vector.tensor_copy(sums[:], csb[64:65, :])
                        prb = s_ps.tile([128, 1024], F32, tag="ps")
                        for mc2 in range(2):
                            nc.tensor.matmul(
                                prb[0:64, bass.ts(mc2, 512)],
                                ones64[:],
                                sums[:, bass.ts(mc2, 512)],
                                start=True, stop=True,
                            )
                        rbs = eps_pool.tile([64, 1024], F32, tag="rbs")
                        nc.vector.reciprocal_approx_fast(rbs[:], prb[0:64, :])
                        nc.vector.tensor_mul(
                            ctx_sb[hp][bass.ts(hl, 64), moff : moff + 1024],
                            csb[0:64, :],
                            rbs[:],
                        )

        # ---- output projection (partial) ----
        with (
            tc.tile_pool(name="o_ps", bufs=3, space="PSUM") as o_ps,
            tc.tile_pool(name="o_sb", bufs=4) as o_sb_pool,
        ):
            for kt in range(2):
                nc.sync.dma_start(wo_sb[:, bass.ts(kt, E)], woT_d[bass.ts(kt, 128), :])
            for mt in range(M // 128):
                for ec in range(2):
                    po = o_ps.tile([128, 512], F32, tag="po")
                    for kt in range(2):
                        nc.tensor.matmul(
                            po[:],
                            ctx_sb[kt][:, bass.ts(mt, 128)],
                            wo_sb[:, kt * E + ec * 512 : kt * E + (ec + 1) * 512],
                            start=(kt == 0), stop=(kt == 1),
                        )
                    ob = o_sb_pool.tile([128, 512], F32, tag="ob")
                    if (mt * 2 + ec) % 2 == 0:
                        nc.vector.tensor_copy(ob[:], po[:])
                    else:
                        nc.scalar.copy(ob[:], po[:])
                    nc.sync.dma_start(
                        out_d[bass.ts(mt, 128), bass.ts(ec, 512)], ob[:]
                    )

    nc.finalize()
    return nc


_PROGRAM = None


def _get_program():
    global _PROGRAM
    if _PROGRAM is None:
        _PROGRAM = build_program()
    return _PROGRAM


def _make_in_maps(q, k, v, mask, Wq, Wk, Wv, Wo):
    q = np.asarray(q, dtype=np.float32)
    k = np.asarray(k, dtype=np.float32)
    v = np.asarray(v, dtype=np.float32)
    mask = np.asarray(mask)
    Wq = np.asarray(Wq, dtype=np.float32)
    Wk = np.asarray(Wk, dtype=np.float32)
    Wv = np.asarray(Wv, dtype=np.float32)
    Wo = np.asarray(Wo, dtype=np.float32)

    per_batch = {}
    for b in range(B):
        per_batch[b] = dict(
            qT=np.ascontiguousarray(q[b].T.astype(np.float16)),
            kT=np.ascontiguousarray(k[b].T.astype(np.float16)),
            vT=np.ascontiguousarray(v[b].T.astype(np.float16)),
            keepT=np.ascontiguousarray(
                np.logical_not(mask[b]).T.astype(np.float16)
            ),
        )

    in_maps = []
    for c in range(NCORES):
        b, hg = divmod(c, GROUPS)
        sl = slice(hg * DLOC, (hg + 1) * DLOC)
        in_maps.append(
            dict(
                per_batch[b],
                wqT=np.ascontiguousarray(Wq[sl].T.astype(np.float16)),
                wkT=np.ascontiguousarray(Wk[sl].T.astype(np.float16)),
                wvT=np.ascontiguousarray(Wv[sl].T.astype(np.float16)),
                woT=np.ascontiguousarray(Wo[:, sl].T.astype(np.float16)),
            )
        )
    return in_maps


def _run(in_maps, trace=False):
    nc = _get_program()
    return run_bass_kernel_spmd(
        nc, in_maps, list(range(NCORES)), trace=trace
    )


def _assemble(results):
    out = np.zeros((B, M, E), dtype=np.float32)
    for c in range(NCORES):
        b = c // GROUPS
        out[b] += results[c]["out"]
    return out


def kernel(q, k, v, mask, Wq, Wk, Wv, Wo):
    in_maps = _make_in_maps(q, k, v, mask, Wq, Wk, Wv, Wo)
    res = _run(in_maps, trace=False)
    return _assemble(res.results)


def run_profiled(q, k, v, mask, Wq, Wk, Wv, Wo):
    """Like kernel(), but traces execution; returns (out, BassKernelResults)."""
    in_maps = _make_in_maps(q, k, v, mask, Wq, Wk, Wv, Wo)
    res = _run(in_maps, trace=True)
    return _assemble(res.results), res
